# revision 62
# baseline (speedup 1.0000x reference)
"""Trainium2 Bass kernel for nn_PixtralHFVisionModel (8-core TP).

Strategy (Megatron tensor-parallel over 8 NeuronCores):
  - Patch-embed conv as matmul (host im2col), replicated on all cores.
  - Activations live TRANSPOSED in SBUF: [hidden(partitions), S(free)];
    residual stream kept in bf16 (output cast to f32 on host).
  - Per-core: 2 of 16 heads (q/k/v column split + row-parallel o-proj)
    and 512 of 4096 MLP intermediate channels. Partial o-proj and
    down-proj outputs are AllReduced (bf16) across the 8 cores, one
    1MB collective per 512-token chunk (24 total). fp8 payloads and
    fp8 matmuls were measured and rejected: quantizing either the
    residual deltas or matmul inputs costs 2-5% relative error vs the
    2e-2 budget (fp8 noise does not average out relative to sums).
  - Attention per image block (1024/512 tokens): block-diag mask is
    free. Scores built transposed [kv, q] so the exp output feeds the
    AV matmul directly; softmax denominator from an appended
    ones-column in the V operand; exp without max subtraction.
    RoPE rotate-half via a constant permutation matmul.
  - RMS stats: one wide squares op + 3 tree adds on DVE (bf16), a
    single ones-matmul for the partition sum, Rsqrt on ACT, and a
    K=1 broadcast matmul for the per-token scale.
  - The schedule is collective-latency-limited (~30-45us per
    AllReduce on the serialized ncfw ring). To keep the ring fed and
    consumers unblocked:
      * per-chunk software pipelining: each chunk's mlp-AR is
        consumed immediately before that chunk's q/k/v work at the
        next layer, and each chunk's o-AR immediately before its mlp,
        so a later chunk's pending AR never sits in front of an
        earlier chunk's compute in any engine queue;
      * image-0 attention is split per q-chunk with the AV psum
        accumulation left open across kv segments, so scores start
        before the second chunk's AR has landed;
      * AR-input staging DMAs use the scalar queue's HARDWARE DGE
        (the gpsimd software-DGE path adds ~15us latency per 1MB
        transfer and was delaying every collective trigger); the
        gpsimd queue carries only collective triggers, AR-output
        reads own the sync queue, weight prefetch is issued one
        layer ahead;
      * the final layer's mlp uses ReduceScatter instead of
        AllReduce: each core ships its 1/8 hidden-stripe of the
        summed delta to a second output and the host does the last
        residual add, removing the full AR + consume + write tail.
All matmuls bf16 inputs, f32 PSUM accumulation.
"""
import sys

if "/opt/trn_rl_repo" not in sys.path:
    sys.path.insert(0, "/opt/trn_rl_repo")

import numpy as np
import ml_dtypes

BF16 = ml_dtypes.bfloat16
NCORES = 8
HID = 1024
HD = 64
INTER = 4096
NLAYERS = 4
PATCH = 16
MAXSIDE = 64
THETA = 10000.0
EPS = 1e-5
SCALE = HD ** -0.5
GRIDS = [(32, 32), (32, 16)]
S0, S1 = 1024, 512
S = S0 + S1
CH = 512            # free-dim matmul chunk (one PSUM bank of f32)
NCH = S // CH       # 3
KT = HID // 128     # 8 hidden k-tiles
PKT = 768 // 128    # 6 patch k-tiles
MT_I = 512 // 128   # 4 intermediate m-tiles per core

_CACHE = {}


def _build_nc():
    import concourse.bacc as bacc
    from concourse import tile
    import concourse.mybir as mybir

    dt = mybir.dt
    f32, bf16 = dt.float32, dt.bfloat16
    fp8 = dt.float8e4
    AF = mybir.ActivationFunctionType
    ALU = mybir.AluOpType

    nc = bacc.Bacc("TRN2", target_bir_lowering=False, debug=False,
                   num_devices=NCORES)

    def din(name, shape, dtype=bf16):
        return nc.dram_tensor(name, shape, dtype, kind="ExternalInput")

    patchesT_d = din("patchesT", [128, PKT, S])
    convWT_d = din("convWT", [128, PKT, HID])
    cos2_d = din("cos2", [128, S])
    sin2_d = din("sin2", [128, S])
    rotP_d = din("rotP", [128, 128])
    lnw_d = din("lnw", [128, KT], f32)
    wq_d = din("wq", [NLAYERS, 128, KT, 128])
    wk_d = din("wk", [NLAYERS, 128, KT, 128])
    wv_d = din("wv", [NLAYERS, 128, KT, 128])
    wo_d = din("wo", [NLAYERS, 128, KT, 128])
    wg_d = din("wg", [NLAYERS, 128, KT, 512])
    wu_d = din("wu", [NLAYERS, 128, KT, 512])
    wd_d = din("wd", [NLAYERS, 128, MT_I, HID])
    out_d = nc.dram_tensor("out", [128, KT, S], bf16,
                           kind="ExternalOutput")
    out2_d = nc.dram_tensor("out2", [16, NCH, KT, CH], bf16,
                            kind="ExternalOutput")

    import concourse.bass as bass_mod

    with tile.TileContext(nc) as tc:
        with (
            tc.tile_pool(name="const", bufs=1) as constp,
            tc.tile_pool(name="big", bufs=1) as bigp,
            tc.tile_pool(name="wat", bufs=2) as watp,
            tc.tile_pool(name="wmlp", bufs=2) as wmlpp,
            tc.tile_pool(name="wdp", bufs=2) as wdp,
            tc.tile_pool(name="att", bufs=1) as attp,
            tc.tile_pool(name="wrk1", bufs=1) as wrk1p,
            tc.tile_pool(name="wrk2", bufs=2) as wrk2p,
            tc.tile_pool(name="cast", bufs=2) as castp,
            tc.tile_pool(name="dram", bufs=2, space="DRAM") as dramp,
            tc.tile_pool(name="psA", bufs=2, space="PSUM") as psA,
            tc.tile_pool(name="psB", bufs=3, space="PSUM") as psB,
            tc.tile_pool(name="psC", bufs=2, space="PSUM") as psC,
            tc.tile_pool(name="psS", bufs=1, space="PSUM") as psS,
        ):
            IMW = [S0, S1]          # tokens per image
            IMO = [0, S0]           # global token offset per image
            NKV = [S0 // 128, S1 // 128]
            # ---- persistent tiles ----
            cos2 = constp.tile([128, S], bf16, tag="cos2")
            sin2 = constp.tile([128, S], bf16, tag="sin2")
            rotP = constp.tile([128, 128], bf16, tag="rotP")
            ones1 = constp.tile([128, 1], bf16, tag="ones1")
            onesr = constp.tile([1, 64], bf16, tag="onesr")
            onesrb = constp.tile([1, 128], bf16, tag="onesrb")
            epsc = constp.tile([128, 1], f32, tag="epsc")
            lnw = constp.tile([128, KT], f32, tag="lnw")
            nc.scalar.dma_start(cos2[:], cos2_d[:])
            nc.scalar.dma_start(sin2[:], sin2_d[:])
            nc.scalar.dma_start(rotP[:], rotP_d[:])
            nc.scalar.dma_start(lnw[:], lnw_d[:])
            nc.gpsimd.memset(ones1[:], 1.0)
            nc.gpsimd.memset(onesr[:], 1.0)
            nc.gpsimd.memset(onesrb[:], 1.0)
            nc.gpsimd.memset(epsc[:], EPS)

            # warmup collective: absorb initial core skew during conv
            warm = constp.tile([128, 8], f32, tag="warm")
            nc.gpsimd.memset(warm[:], 1.0)
            warm_i = dramp.tile([128, 8], f32, tag="warm_i")
            warm_o = dramp.tile([128, 8], f32, tag="warm_o",
                                addr_space="Shared")
            nc.gpsimd.dma_start(warm_i[:], warm[:])
            nc.gpsimd.collective_compute(
                "AllReduce", ALU.add, ins=[warm_i.opt()], outs=[warm_o.opt()],
                replica_groups=[list(range(NCORES))])

            def act_raw(out, in_, func, bias=0.0, scale=1.0):
                """activation() without the Rsqrt/Reciprocal accuracy guard."""
                eng = nc.scalar
                inputs = [eng.lower_ap(in_)]
                for arg in (bias, scale, 0.0):
                    if isinstance(arg, bass_mod.AP):
                        inputs.append(eng.lower_ap(arg))
                    else:
                        inputs.append(mybir.ImmediateValue(
                            dtype=f32, value=float(arg)))
                return eng.add_instruction(mybir.InstActivation(
                    name=f"I-{nc.next_id()}", func=func,
                    ins=inputs, outs=[eng.lower_ap(out)]))

            # per-image single residual / normed tiles [128, KT, W]:
            # wide multi-ktile DVE ops amortize per-op overhead
            resids = [bigp.tile([128, KT, IMW[i]], bf16, tag=f"res{i}",
                                name=f"res{i}") for i in range(2)]
            xnorms = [bigp.tile([128, KT, IMW[i]], bf16, tag=f"xn{i}",
                                name=f"xn{i}") for i in range(2)]
            hmlps = [[bigp.tile([128, IMW[i]], bf16, tag=f"hm{i}_{m}",
                                name=f"hm{i}_{m}") for m in range(MT_I)]
                     for i in range(2)]
            # persistent V tiles: [kv-token(part), kvblk, 2*(64+1)] with the
            # ones (denominator) columns written once
            v2s = [bigp.tile([128, NKV[i], 130], bf16, tag=f"v2_{i}",
                             name=f"v2_{i}") for i in range(2)]
            for i in range(2):
                for kv in range(NKV[i]):
                    nc.gpsimd.memset(v2s[i][:, kv, 64:65], 1.0)
                    nc.gpsimd.memset(v2s[i][:, kv, 129:130], 1.0)

            # round-robin psum->sbuf evacuation across engines
            _rr = [0]

            def evac(dst, src):
                e = _rr[0] = (_rr[0] + 1) % 2
                if e == 0:
                    nc.scalar.activation(dst, src, AF.Copy)
                    return nc.scalar
                nc.vector.tensor_copy(dst, src)
                return nc.vector

            def rms_tail(img, co, rstd0, pss, wcol=None,
                         write_back_f32=False):
                csl = slice(co, co + CH)
                act_raw(rstd0[:, csl], pss[:], AF.Rsqrt,
                        bias=epsc[0:1, :], scale=1.0 / HID)
                rstdb = psB.tile([128, 512], f32, tag="psb", name="rstdb")
                nc.tensor.matmul(rstdb[:, 0:CH], lhsT=onesrb[:],
                                 rhs=rstd0[0:1, csl], start=True, stop=True)
                rsb = castp.tile([128, 1, CH], bf16, tag="rsb", bufs=2)
                evac(rsb[:, 0, :], rstdb[:, 0:CH])
                if wcol is None:
                    a1, a2 = bass_mod.broadcast_tensor_aps(
                        resids[img][:, :, csl], rsb[:])
                    nc.vector.tensor_mul(xnorms[img][:, :, csl], a1, a2)
                else:
                    for kt in range(KT):
                        nc.vector.scalar_tensor_tensor(
                            xnorms[img][:, kt, csl],
                            resids[img][:, kt, csl],
                            wcol[:, kt:kt + 1], rsb[:, 0, :],
                            ALU.mult, ALU.mult)
                if write_back_f32:
                    nc.scalar.activation(resids[img][:, :, csl],
                                         xnorms[img][:, :, csl],
                                         AF.Copy)

            def rms_chunk(img, co, rstd0, wcol=None,
                          write_back_f32=False):
                csl = slice(co, co + CH)
                pss = psS.tile([1, CH], f32, tag="pss")
                sq = castp.tile([128, KT, CH], bf16, tag="sq", bufs=1)
                nc.vector.tensor_mul(sq[:, 0:4, :],
                                     resids[img][:, 0:4, csl],
                                     resids[img][:, 0:4, csl])
                nc.vector.tensor_add(sq[:, 0:2, :], sq[:, 0:2, :],
                                     sq[:, 2:4, :])
                nc.vector.tensor_mul(sq[:, 4:8, :],
                                     resids[img][:, 4:8, csl],
                                     resids[img][:, 4:8, csl])
                nc.vector.tensor_add(sq[:, 4:6, :], sq[:, 4:6, :],
                                     sq[:, 6:8, :])
                nc.vector.tensor_add(sq[:, 0:2, :], sq[:, 0:2, :],
                                     sq[:, 4:6, :])
                nc.vector.tensor_add(sq[:, 0, :], sq[:, 0, :], sq[:, 1, :])
                nc.tensor.matmul(pss[:], lhsT=ones1[:], rhs=sq[:, 0, :],
                                 start=True, stop=True)
                rms_tail(img, co, rstd0, pss, wcol, write_back_f32)

            def rms_norm(img, wcol=None, write_back_f32=False):
                W = IMW[img]
                rstd0 = wrk1p.tile([1, W], bf16, tag=f"rstd0{img}")
                for co in range(0, W, CH):
                    rms_chunk(img, co, rstd0, wcol, write_back_f32)

            # ---- conv patch embed (replicated, streamed) + ln_pre ----
            with tc.tile_pool(name="convp", bufs=2) as convp:
                rstd0s = [wrk1p.tile([1, IMW[i]], bf16, tag=f"rstd0{i}",
                                     name=f"rstd0c{i}")
                          for i in range(2)]
                for chi in range(NCH):
                    gco = chi * CH
                    img = 0 if gco < S0 else 1
                    lco = gco - IMO[img]
                    pch = convp.tile([128, PKT, CH], bf16, tag="pch")
                    nc.sync.dma_start(pch[:], patchesT_d[:, :, gco:gco + CH])
                    for kt in range(KT):
                        cwt = convp.tile([128, PKT, 128], bf16, tag="cwt")
                        nc.sync.dma_start(
                            cwt[:], convWT_d[:, :, kt * 128:(kt + 1) * 128])
                        psx = psA.tile([128, CH], f32, tag="psx")
                        for pk in range(PKT):
                            nc.tensor.matmul(
                                psx[:], lhsT=cwt[:, pk, :],
                                rhs=pch[:, pk, :],
                                start=(pk == 0), stop=(pk == PKT - 1))
                        nc.scalar.activation(
                            resids[img][:, kt, lco:lco + CH], psx[:],
                            AF.Copy)
                    # ln_pre for this chunk rides under the next chunk's
                    # conv matmuls instead of running serially after all
                    # of conv
                    rms_chunk(img, lco, rstd0s[img], lnw,
                              write_back_f32=True)

            def qkv_attn(img, wq, wk, wv, wo, pend_in,
                         lnw_post=None):
                """Full attention for one image -> AR output dram tile."""
                W = IMW[img]
                lo = IMO[img]
                nq = W // CH
                nkv = NKV[img]
                v2 = v2s[img]
                qt = attp.tile([128, W], bf16, tag=f"qt{img}")
                kt_t = attp.tile([128, W], bf16, tag=f"kt{img}")
                otcs = [attp.tile([128, CH], bf16, tag=f"otc{img}_{ci}",
                                  name=f"otc{img}_{ci}") for ci in range(nq)]
                arouts = []

                def oproj(ci):
                    # row-parallel o-projection: this core's 2 heads (128
                    # rows) x full o-weight slice -> partial [HID, CH],
                    # AllReduced per chunk
                    arin = dramp.tile([128, KT, CH], bf16,
                                      tag=f"coi{img}{ci}",
                                      name=f"coi{img}{ci}")
                    aro = dramp.tile([128, KT, CH], bf16,
                                     tag=f"coo{img}{ci}",
                                     name=f"coo{img}{ci}",
                                     addr_space="Shared")
                    stage = stgp.tile([128, KT, CH], bf16, tag="stgo",
                                      bufs=1)
                    for kt in range(KT):
                        pso = psA.tile([128, CH], f32, tag="psx")
                        nc.tensor.matmul(pso[:], lhsT=wo[:, kt, :],
                                         rhs=otcs[ci][:],
                                         start=True, stop=True)
                        evac(stage[:, kt, :], pso[:])
                    nc.scalar.dma_start(arin[:], stage[:])
                    nc.gpsimd.collective_compute(
                        "AllReduce", ALU.add,
                        ins=[arin.opt()], outs=[aro.opt()],
                        replica_groups=[list(range(NCORES))])
                    arouts.append((aro, ci * CH))

                def qkrope_chunk(co):
                    # q/k projection + rope + v blocks for ONE token chunk
                    # (lets attention on earlier chunks start while later
                    # chunks still wait on their mlp-AllReduce)
                    csl = slice(co, co + CH)
                    gsl = slice(lo + co, lo + co + CH)
                    for dst, w in ((qt, wq), (kt_t, wk)):
                        psq = psA.tile([128, CH], f32, tag="psx")
                        for kt in range(KT):
                            nc.tensor.matmul(
                                psq[:], lhsT=w[:, kt, :],
                                rhs=xnorms[img][:, kt, csl],
                                start=(kt == 0), stop=(kt == KT - 1))
                        nc.scalar.activation(dst[:, csl], psq[:], AF.Copy)
                    for dst in (qt, kt_t):
                        psr = psB.tile([128, 512], f32, tag="psb")
                        nc.tensor.matmul(psr[:, 0:CH], lhsT=rotP[:],
                                         rhs=dst[:, csl],
                                         start=True, stop=True)
                        t1 = castp.tile([128, CH], bf16, tag="t1", bufs=2)
                        t2 = castp.tile([128, CH], bf16, tag="t2", bufs=2)
                        nc.vector.tensor_mul(t1[:], dst[:, csl],
                                             cos2[:, gsl])
                        nc.vector.tensor_mul(t2[:], psr[:, 0:CH],
                                             sin2[:, gsl])
                        nc.vector.tensor_add(dst[:, csl], t1[:], t2[:])
                    for kv in range(co // 128, (co + CH) // 128):
                        psv = psB.tile([128, 512], f32, tag="psb")
                        for kt in range(KT):
                            nc.tensor.matmul(
                                psv[:, 0:128],
                                lhsT=xnorms[img][:, kt,
                                                 kv * 128:(kv + 1) * 128],
                                rhs=wv[:, kt, :],
                                start=(kt == 0), stop=(kt == KT - 1))
                        nc.vector.tensor_copy(v2[:, kv, 0:64], psv[:, 0:64])
                        nc.scalar.activation(v2[:, kv, 65:129],
                                             psv[:, 64:128], AF.Copy)

                psavs_by = {}
                pts_by = {}

                def issue_scores(ci, i):
                    qsl = slice(ci * CH, (ci + 1) * CH)
                    for h in range(2):
                        hsl = slice(h * 64, (h + 1) * 64)
                        pss = psB.tile([128, 512], f32, tag="psb")
                        nc.tensor.matmul(
                            pss[:, 0:CH],
                            lhsT=kt_t[hsl, i * 128:(i + 1) * 128],
                            rhs=qt[hsl, qsl], start=True, stop=True)
                        pt = castp.tile([128, CH], bf16, tag="pt",
                                        bufs=4, name=f"pt{h}")
                        nc.scalar.activation(pt[:], pss[:, 0:CH],
                                             AF.Exp, scale=SCALE)
                        pts_by[(ci, h, i)] = pt

                def att_seg(ci, a, b):
                    # scores+AV for q-chunk ci against kv blocks [a, b);
                    # the AV psum accumulation stays open across segments
                    if ci not in psavs_by:
                        psavs_by[ci] = [psC.tile([65, CH], f32, tag="psav",
                                                 name=f"psav{h}")
                                        for h in range(2)]
                    psavs = psavs_by[ci]
                    issue_scores(ci, a)
                    for i in range(a, b):
                        if i + 1 < b:
                            issue_scores(ci, i + 1)
                        for h in range(2):
                            nc.tensor.matmul(
                                psavs[h][:],
                                lhsT=v2[:, i, h * 65:h * 65 + 65],
                                rhs=pts_by.pop((ci, h, i))[:],
                                start=(i == 0), stop=(i == nkv - 1))

                def fin(ci):
                    # softmax denominators + output scaling + o-projection
                    psavs = psavs_by.pop(ci)
                    psbc = psB.tile([128, 512], f32, tag="psb")
                    for h in range(2):
                        rec = castp.tile([1, CH], bf16, tag="rec", bufs=2)
                        with nc.allow_low_precision("softmax denom bf16"):
                            nc.vector.reciprocal(rec[:], psavs[h][64:65, :])
                        nc.tensor.matmul(
                            psbc[h * 64:(h + 1) * 64, 0:CH],
                            lhsT=onesr[:], rhs=rec[:],
                            start=True, stop=True, skip_group_check=True)
                    obc = castp.tile([128, CH], bf16, tag="obc", bufs=2)
                    nc.vector.tensor_copy(obc[:], psbc[:, 0:CH])
                    for h in range(2):
                        nc.vector.tensor_mul(
                            otcs[ci][h * 64:(h + 1) * 64, :],
                            psavs[h][0:64, :], obc[h * 64:(h + 1) * 64, :])
                    oproj(ci)

                rstd0 = wrk1p.tile([1, W], bf16, tag=f"rstd0{img}")

                def consume(ci):
                    # consume this chunk's pending mlp-AllReduce (or do the
                    # initial rms) right before the chunk's q/k/v work, so
                    # later chunks' ARs are not in front of earlier chunks'
                    # attention in any queue
                    if pend_in is not None:
                        aro, co = pend_in[ci]
                        consume_chunk(img, aro, co, rstd0)
                    else:
                        rms_chunk(img, ci * CH, rstd0, lnw_post)

                if nq == 1:
                    consume(0)
                    qkrope_chunk(0)
                    att_seg(0, 0, nkv)
                    fin(0)
                else:
                    consume(0)
                    qkrope_chunk(0)
                    att_seg(0, 0, 4)
                    consume(1)
                    qkrope_chunk(CH)
                    att_seg(0, 4, 8)
                    fin(0)
                    att_seg(1, 0, 8)
                    fin(1)
                return arouts

            def consume_chunk(img, aro, co, rstd0, write_out=False):
                lo = IMO[img]
                csl = slice(co, co + CH)
                arr = wrk2p.tile([128, KT, CH], bf16, tag="arrc",
                                 bufs=1)
                H = KT // 2
                nc.sync.dma_start(arr[:, 0:H, :], aro[:, 0:H, :])
                nc.sync.dma_start(arr[:, H:KT, :], aro[:, H:KT, :])
                nc.vector.tensor_add(resids[img][:, 0:H, csl],
                                     resids[img][:, 0:H, csl],
                                     arr[:, 0:H, :])
                nc.vector.tensor_add(resids[img][:, H:KT, csl],
                                     resids[img][:, H:KT, csl],
                                     arr[:, H:KT, :])
                if write_out:
                    nc.sync.dma_start(
                        out_d[:, :, lo + co:lo + co + CH],
                        resids[img][:, :, csl])
                if rstd0 is not None:
                    rms_chunk(img, co, rstd0)

            def add_ar(img, arouts, rms=False, write_out=False):
                W = IMW[img]
                rstd0 = None
                if rms:
                    rstd0 = wrk1p.tile([1, W], bf16, tag=f"rstd0{img}")
                for aro, co in arouts:
                    consume_chunk(img, aro, co, rstd0, write_out)

            def mlp(img, wg, wu, wd, ar_o, final=False):
                W = IMW[img]
                lo = IMO[img]
                arouts = []
                rstd0 = wrk1p.tile([1, W], bf16, tag=f"rstd0{img}")
                for co in range(0, W, CH):
                    ci = co // CH
                    # this chunk's o-AllReduce lands here; later chunks'
                    # o-ARs stay behind this chunk's mlp in every queue
                    aro_o, co_o = ar_o[ci]
                    consume_chunk(img, aro_o, co_o, rstd0)
                    if final:
                        # pre-mlp residual out now (hidden under the mlp
                        # compute); host adds the ReduceScattered delta
                        nc.sync.dma_start(
                            out_d[:, :, lo + co:lo + co + CH],
                            resids[img][:, :, co:co + CH])
                    csl = slice(co, co + CH)
                    for mt in range(MT_I):
                        msl = slice(mt * 128, (mt + 1) * 128)
                        psg = psA.tile([128, CH], f32, tag="psx")
                        for kt in range(KT):
                            nc.tensor.matmul(
                                psg[:], lhsT=wg[:, kt, msl],
                                rhs=xnorms[img][:, kt, csl],
                                start=(kt == 0), stop=(kt == KT - 1))
                        gts = castp.tile([128, CH], bf16, tag="gts")
                        nc.scalar.activation(gts[:], psg[:], AF.Silu)
                        psu = psB.tile([128, 512], f32, tag="psb")
                        for kt in range(KT):
                            nc.tensor.matmul(
                                psu[:, 0:CH], lhsT=wu[:, kt, msl],
                                rhs=xnorms[img][:, kt, csl],
                                start=(kt == 0), stop=(kt == KT - 1))
                        nc.vector.tensor_mul(hmlps[img][mt][:, csl], gts[:],
                                             psu[:, 0:CH])
                    arin = dramp.tile([128, KT, CH], bf16,
                                      tag=f"cmi{img}{ci}",
                                      name=f"cmi{img}{ci}")
                    aro = dramp.tile([128, KT, CH], bf16,
                                     tag=f"cmo{img}{ci}",
                                     name=f"cmo{img}{ci}",
                                     addr_space="Shared")
                    stage = stgp.tile([128, KT, CH], bf16, tag="stg",
                                      bufs=2)
                    for kt in range(KT):
                        psd = psA.tile([128, CH], f32, tag="psx")
                        for mt in range(MT_I):
                            nc.tensor.matmul(
                                psd[:],
                                lhsT=wd[:, mt, kt * 128:(kt + 1) * 128],
                                rhs=hmlps[img][mt][:, co:co + CH],
                                start=(mt == 0), stop=(mt == MT_I - 1))
                        evac(stage[:, kt, :], psd[:])
                    nc.scalar.dma_start(arin[:], stage[:])
                    if final:
                        gi = ci if img == 0 else 2
                        rso = dramp.tile([16, KT, CH], bf16,
                                         tag=f"rso{img}{ci}",
                                         name=f"rso{img}{ci}")
                        nc.gpsimd.collective_compute(
                            "ReduceScatter", ALU.add,
                            ins=[arin.opt()], outs=[rso.opt()],
                            replica_groups=[list(range(NCORES))])
                        nc.sync.dma_start(out2_d[:, gi], rso[:])
                    else:
                        nc.gpsimd.collective_compute(
                            "AllReduce", ALU.add,
                            ins=[arin.opt()], outs=[aro.opt()],
                            replica_groups=[list(range(NCORES))])
                        arouts.append((aro, co))
                return arouts

            # ---- transformer layers, software-pipelined across the MLP
            # AllReduce: layer l's MLP AR for image i is added at the top of
            # layer l+1 right before that image's attention norm. Image 1
            # (the small one) goes first so its AllReduce hides under image
            # 0's larger compute ----
            with tc.tile_pool(name="stg", bufs=2) as stgp:
                wts = {}

                def load_weights(l):
                    # weight DMAs ride the gpsimd queue: it carries only
                    # AR-input writes and collective triggers, so these
                    # never sit behind a blocking AR-output wait
                    wq = watp.tile([128, KT, 128], bf16, tag="wq")
                    wk = watp.tile([128, KT, 128], bf16, tag="wk")
                    wv = watp.tile([128, KT, 128], bf16, tag="wv")
                    wo = watp.tile([128, KT, 128], bf16, tag="wo")
                    wg = wmlpp.tile([128, KT, 512], bf16, tag="wg")
                    wu = wmlpp.tile([128, KT, 512], bf16, tag="wu")
                    wd = wdp.tile([128, MT_I, HID], bf16, tag="wd")
                    for t, d in ((wq, wq_d), (wk, wk_d), (wv, wv_d),
                                 (wo, wo_d), (wg, wg_d), (wu, wu_d),
                                 (wd, wd_d)):
                        nc.sync.dma_start(t[:], d[l])
                    wts[l] = (wq, wk, wv, wo, wg, wu, wd)

                load_weights(0)
                pend = [None, None]
                for l in range(NLAYERS):
                    wq, wk, wv, wo, wg, wu, wd = wts.pop(l)
                    ar_a = [None, None]
                    for img in (1, 0):
                        ar_a[img] = qkv_attn(img, wq, wk, wv, wo,
                                             pend[img])
                        pend[img] = None
                    if l + 1 < NLAYERS:
                        load_weights(l + 1)
                    for img in (1, 0):
                        pend[img] = mlp(img, wg, wu, wd, ar_a[img],
                                        final=(l == NLAYERS - 1))

    nc.compile()
    return nc


# ---------------- host-side prep ----------------

def _im2col(img):
    C, H, W = img.shape
    h, w = H // PATCH, W // PATCH
    p = img.reshape(C, h, PATCH, w, PATCH).transpose(1, 3, 0, 2, 4)
    return p.reshape(h * w, C * PATCH * PATCH)


def _rope_tables():
    freqs = 1.0 / THETA ** (np.arange(0, HD, 2, dtype=np.float64) / HD)
    fh = np.outer(np.arange(MAXSIDE, dtype=np.float64), freqs[::2])
    fw = np.outer(np.arange(MAXSIDE, dtype=np.float64), freqs[1::2])
    pids = np.concatenate([
        (np.arange(h)[:, None] * MAXSIDE + np.arange(w)[None, :]).reshape(-1)
        for h, w in GRIDS])
    inv = np.concatenate([
        np.broadcast_to(fh[:, None, :], (MAXSIDE, MAXSIDE, HD // 4)),
        np.broadcast_to(fw[None, :, :], (MAXSIDE, MAXSIDE, HD // 4))],
        axis=-1).reshape(-1, HD // 2)
    inv = np.concatenate([inv, inv], axis=-1)
    emb = inv[pids]                                   # [S, 64]
    cosT = np.cos(emb).T.astype(np.float32)           # [64, S]
    sinT = np.sin(emb).T.astype(np.float32)
    sinTs = np.concatenate([-sinT[:32], sinT[32:]], axis=0)
    cos2 = np.concatenate([cosT, cosT], axis=0).astype(BF16)
    sin2 = np.concatenate([sinTs, sinTs], axis=0).astype(BF16)
    return np.ascontiguousarray(cos2), np.ascontiguousarray(sin2)


def _rot_perm():
    """rot[m] = q[perm(m)] permutation as a [k, m] matmul constant."""
    P = np.zeros((128, 128), np.float32)
    for b in (0, 64):
        for m in range(32):
            P[b + 32 + m, b + m] = 1.0          # rot[m] = q[m+32]
            P[b + m, b + 32 + m] = 1.0          # rot[m+32] = q[m]
    return P.astype(BF16)


def _ktile(a, last):
    """[L, 1024, last] -> [L, 128, kt, last] (partition-major k-tiles)."""
    L = a.shape[0]
    return np.ascontiguousarray(
        a.reshape(L, -1, 128, last).transpose(0, 2, 1, 3))


def _prep(inputs):
    f32 = np.float32
    patches = np.concatenate([
        _im2col(np.asarray(inputs["img0"], f32)),
        _im2col(np.asarray(inputs["img1"], f32))])          # [S, 768]
    patchesT = np.ascontiguousarray(
        patches.T.reshape(PKT, 128, S).transpose(1, 0, 2)).astype(BF16)
    cw = np.asarray(inputs["conv_w"], f32).reshape(HID, 768)
    convWT = np.ascontiguousarray(
        cw.T.reshape(PKT, 128, HID).transpose(1, 0, 2)).astype(BF16)
    cos2, sin2 = _rope_tables()
    lnw = np.ascontiguousarray(
        np.asarray(inputs["ln_pre_w"], f32).reshape(KT, 128).T)

    anw = np.asarray(inputs["attn_norm_w"], f32)[:, :, None]  # [4, in, 1]
    fnw = np.asarray(inputs["ffn_norm_w"], f32)[:, :, None]
    qwT = np.asarray(inputs["q_w"], f32).transpose(0, 2, 1) * anw
    kwT = np.asarray(inputs["k_w"], f32).transpose(0, 2, 1) * anw
    vwT = np.asarray(inputs["v_w"], f32).transpose(0, 2, 1) * anw
    owT = np.asarray(inputs["o_w"], f32).transpose(0, 2, 1)   # [4, d, e]
    gwT = np.asarray(inputs["gate_w"], f32).transpose(0, 2, 1) * fnw
    uwT = np.asarray(inputs["up_w"], f32).transpose(0, 2, 1) * fnw
    dwT = np.asarray(inputs["down_w"], f32).transpose(0, 2, 1)  # [4, I, out]

    common = dict(patchesT=patchesT, convWT=convWT, cos2=cos2, sin2=sin2,
                  rotP=_rot_perm(), lnw=lnw)
    in_maps = []
    for c in range(NCORES):
        esl = slice(c * 128, (c + 1) * 128)
        isl = slice(c * 512, (c + 1) * 512)
        m = dict(
            wq=_ktile(qwT[:, :, esl].astype(BF16), 128),
            wk=_ktile(kwT[:, :, esl].astype(BF16), 128),
            wv=_ktile(vwT[:, :, esl].astype(BF16), 128),
            wo=np.ascontiguousarray(
                owT[:, esl, :].reshape(NLAYERS, 128, KT, 128)).astype(BF16),
            wg=_ktile(gwT[:, :, isl].astype(BF16), 512),
            wu=_ktile(uwT[:, :, isl].astype(BF16), 512),
            wd=np.ascontiguousarray(
                dwT[:, isl, :].reshape(NLAYERS, MT_I, 128, HID)
                .transpose(0, 2, 1, 3)).astype(BF16),
            **common)
        in_maps.append(m)
    return in_maps


LAST_RESULTS = None
TRACE = False


def _install_ntff_hook():
    """The RL container's antenv lacks axon_hooks; recreate it so
    trace=True can capture NTFF profiles through the axon terminal."""
    import types
    import antenv

    if hasattr(antenv, "axon_hooks"):
        return
    mod = types.ModuleType("antenv.axon_hooks")
    holder = [None]
    mod.set_axon_ntff_profile_hook = lambda h: holder.__setitem__(0, h)
    mod.get_axon_ntff_profile_hook = lambda: holder[0]
    sys.modules["antenv.axon_hooks"] = mod
    antenv.axon_hooks = mod
    if "/root/.axon_site" not in sys.path:
        sys.path.insert(0, "/root/.axon_site")
    try:
        from trn_agent_boot.trn_boot import _ntff_profile_via_ctypes
        mod.set_axon_ntff_profile_hook(
            _ntff_profile_via_ctypes("/opt/axon/libaxon_pjrt.so"))
    except Exception as e:  # pragma: no cover
        print("ntff hook install failed:", e)


def kernel(**inputs):
    global LAST_RESULTS
    from concourse import bass_utils

    if TRACE:
        _install_ntff_hook()

    if "nc" not in _CACHE:
        _CACHE["nc"] = _build_nc()
    nc = _CACHE["nc"]
    in_maps = _prep(inputs)
    res = bass_utils.run_bass_kernel_spmd(
        nc, in_maps, core_ids=list(range(NCORES)), trace=TRACE)
    LAST_RESULTS = res
    out = res.results[0]["out"]          # [128, KT, S] pre-mlp residual
    full = out.transpose(1, 0, 2).reshape(HID, S).astype(np.float32)
    # final-layer mlp delta arrives ReduceScattered: core c holds hidden
    # rows [c*16, (c+1)*16) of every k-tile
    d = np.zeros((KT, 128, NCH * CH), np.float32)
    for c in range(NCORES):
        o2 = np.asarray(res.results[c]["out2"], np.float32)
        d[:, c * 16:(c + 1) * 16] = (
            o2.transpose(2, 0, 1, 3).reshape(KT, 16, NCH * CH))
    full += d.reshape(HID, NCH * CH)
    return np.ascontiguousarray(full.T[None]).astype(np.float32)



# revision 64
# speedup vs baseline: 1.0029x; 1.0029x over previous
"""Trainium2 Bass kernel for nn_PixtralHFVisionModel (8-core TP).

Strategy (Megatron tensor-parallel over 8 NeuronCores):
  - Patch-embed conv as matmul (host im2col), replicated on all cores.
  - Activations live TRANSPOSED in SBUF: [hidden(partitions), S(free)];
    residual stream kept in bf16 (output cast to f32 on host).
  - Per-core: 2 of 16 heads (q/k/v column split + row-parallel o-proj)
    and 512 of 4096 MLP intermediate channels. Partial o-proj and
    down-proj outputs are AllReduced (bf16) across the 8 cores, one
    1MB collective per 512-token chunk (24 total). fp8 payloads and
    fp8 matmuls were measured and rejected: quantizing either the
    residual deltas or matmul inputs costs 2-5% relative error vs the
    2e-2 budget (fp8 noise does not average out relative to sums).
  - Attention per image block (1024/512 tokens): block-diag mask is
    free. Scores built transposed [kv, q] so the exp output feeds the
    AV matmul directly; softmax denominator from an appended
    ones-column in the V operand; exp without max subtraction.
    RoPE rotate-half via a constant permutation matmul.
  - RMS stats: one wide squares op + 3 tree adds on DVE (bf16), a
    single ones-matmul for the partition sum, Rsqrt on ACT, and a
    K=1 broadcast matmul for the per-token scale.
  - The schedule is collective-latency-limited (~30-45us per
    AllReduce on the serialized ncfw ring). To keep the ring fed and
    consumers unblocked:
      * per-chunk software pipelining: each chunk's mlp-AR is
        consumed immediately before that chunk's q/k/v work at the
        next layer, and each chunk's o-AR immediately before its mlp,
        so a later chunk's pending AR never sits in front of an
        earlier chunk's compute in any engine queue;
      * image-0 attention is split per q-chunk with the AV psum
        accumulation left open across kv segments, so scores start
        before the second chunk's AR has landed;
      * AR-input staging DMAs use the scalar queue's HARDWARE DGE
        (the gpsimd software-DGE path adds ~15us latency per 1MB
        transfer and was delaying every collective trigger); the
        gpsimd queue carries only collective triggers, AR-output
        reads own the sync queue, weight prefetch is issued one
        layer ahead;
      * the final layer's mlp uses ReduceScatter instead of
        AllReduce: each core ships its 1/8 hidden-stripe of the
        summed delta to a second output and the host does the last
        residual add, removing the full AR + consume + write tail.
All matmuls bf16 inputs, f32 PSUM accumulation.
"""
import sys

if "/opt/trn_rl_repo" not in sys.path:
    sys.path.insert(0, "/opt/trn_rl_repo")

import numpy as np
import ml_dtypes

BF16 = ml_dtypes.bfloat16
NCORES = 8
HID = 1024
HD = 64
INTER = 4096
NLAYERS = 4
PATCH = 16
MAXSIDE = 64
THETA = 10000.0
EPS = 1e-5
SCALE = HD ** -0.5
GRIDS = [(32, 32), (32, 16)]
S0, S1 = 1024, 512
S = S0 + S1
CH = 512            # free-dim matmul chunk (one PSUM bank of f32)
NCH = S // CH       # 3
KT = HID // 128     # 8 hidden k-tiles
PKT = 768 // 128    # 6 patch k-tiles
MT_I = 512 // 128   # 4 intermediate m-tiles per core

_CACHE = {}


def _build_nc():
    import concourse.bacc as bacc
    from concourse import tile
    import concourse.mybir as mybir

    dt = mybir.dt
    f32, bf16 = dt.float32, dt.bfloat16
    fp8 = dt.float8e4
    AF = mybir.ActivationFunctionType
    ALU = mybir.AluOpType

    nc = bacc.Bacc("TRN2", target_bir_lowering=False, debug=False,
                   num_devices=NCORES)

    def din(name, shape, dtype=bf16):
        return nc.dram_tensor(name, shape, dtype, kind="ExternalInput")

    patchesT_d = din("patchesT", [128, PKT, S])
    convWT_d = din("convWT", [128, PKT, HID])
    cos2_d = din("cos2", [128, S])
    sin2_d = din("sin2", [128, S])
    rotP_d = din("rotP", [128, 128])
    lnw_d = din("lnw", [128, KT], f32)
    wq_d = din("wq", [NLAYERS, 128, KT, 128])
    wk_d = din("wk", [NLAYERS, 128, KT, 128])
    wv_d = din("wv", [NLAYERS, 128, KT, 128])
    wo_d = din("wo", [NLAYERS, 128, KT, 128])
    wg_d = din("wg", [NLAYERS, 128, KT, 512])
    wu_d = din("wu", [NLAYERS, 128, KT, 512])
    wd_d = din("wd", [NLAYERS, 128, MT_I, HID])
    out_d = nc.dram_tensor("out", [128, KT, S], bf16,
                           kind="ExternalOutput")
    out2_d = nc.dram_tensor("out2", [16, NCH, KT, CH], bf16,
                            kind="ExternalOutput")

    import concourse.bass as bass_mod

    with tile.TileContext(nc) as tc:
        with (
            tc.tile_pool(name="const", bufs=1) as constp,
            tc.tile_pool(name="big", bufs=1) as bigp,
            tc.tile_pool(name="wat", bufs=2) as watp,
            tc.tile_pool(name="wmlp", bufs=2) as wmlpp,
            tc.tile_pool(name="wdp", bufs=2) as wdp,
            tc.tile_pool(name="att", bufs=1) as attp,
            tc.tile_pool(name="wrk1", bufs=1) as wrk1p,
            tc.tile_pool(name="wrk2", bufs=2) as wrk2p,
            tc.tile_pool(name="cast", bufs=2) as castp,
            tc.tile_pool(name="dram", bufs=2, space="DRAM") as dramp,
            tc.tile_pool(name="psA", bufs=2, space="PSUM") as psA,
            tc.tile_pool(name="psB", bufs=3, space="PSUM") as psB,
            tc.tile_pool(name="psC", bufs=2, space="PSUM") as psC,
            tc.tile_pool(name="psS", bufs=1, space="PSUM") as psS,
        ):
            IMW = [S0, S1]          # tokens per image
            IMO = [0, S0]           # global token offset per image
            NKV = [S0 // 128, S1 // 128]
            # ---- persistent tiles ----
            cos2 = constp.tile([128, S], bf16, tag="cos2")
            sin2 = constp.tile([128, S], bf16, tag="sin2")
            rotP = constp.tile([128, 128], bf16, tag="rotP")
            ones1 = constp.tile([128, 1], bf16, tag="ones1")
            onesr = constp.tile([1, 64], bf16, tag="onesr")
            onesrb = constp.tile([1, 128], bf16, tag="onesrb")
            epsc = constp.tile([128, 1], f32, tag="epsc")
            lnw = constp.tile([128, KT], f32, tag="lnw")
            nc.scalar.dma_start(cos2[:], cos2_d[:])
            nc.scalar.dma_start(sin2[:], sin2_d[:])
            nc.scalar.dma_start(rotP[:], rotP_d[:])
            nc.scalar.dma_start(lnw[:], lnw_d[:])
            nc.gpsimd.memset(ones1[:], 1.0)
            nc.gpsimd.memset(onesr[:], 1.0)
            nc.gpsimd.memset(onesrb[:], 1.0)
            nc.gpsimd.memset(epsc[:], EPS)

            # warmup collective: absorb initial core skew during conv
            warm = constp.tile([128, 8], f32, tag="warm")
            nc.gpsimd.memset(warm[:], 1.0)
            warm_i = dramp.tile([128, 8], f32, tag="warm_i")
            warm_o = dramp.tile([128, 8], f32, tag="warm_o",
                                addr_space="Shared")
            nc.gpsimd.dma_start(warm_i[:], warm[:])
            nc.gpsimd.collective_compute(
                "AllReduce", ALU.add, ins=[warm_i.opt()], outs=[warm_o.opt()],
                replica_groups=[list(range(NCORES))])

            def act_raw(out, in_, func, bias=0.0, scale=1.0):
                """activation() without the Rsqrt/Reciprocal accuracy guard."""
                eng = nc.scalar
                inputs = [eng.lower_ap(in_)]
                for arg in (bias, scale, 0.0):
                    if isinstance(arg, bass_mod.AP):
                        inputs.append(eng.lower_ap(arg))
                    else:
                        inputs.append(mybir.ImmediateValue(
                            dtype=f32, value=float(arg)))
                return eng.add_instruction(mybir.InstActivation(
                    name=f"I-{nc.next_id()}", func=func,
                    ins=inputs, outs=[eng.lower_ap(out)]))

            # per-image single residual / normed tiles [128, KT, W]:
            # wide multi-ktile DVE ops amortize per-op overhead
            resids = [bigp.tile([128, KT, IMW[i]], bf16, tag=f"res{i}",
                                name=f"res{i}") for i in range(2)]
            xnorms = [bigp.tile([128, KT, IMW[i]], bf16, tag=f"xn{i}",
                                name=f"xn{i}") for i in range(2)]
            hmlps = [[bigp.tile([128, IMW[i]], bf16, tag=f"hm{i}_{m}",
                                name=f"hm{i}_{m}") for m in range(MT_I)]
                     for i in range(2)]
            # persistent V tiles: [kv-token(part), kvblk, 2*(64+1)] with the
            # ones (denominator) columns written once
            v2s = [bigp.tile([128, NKV[i], 130], bf16, tag=f"v2_{i}",
                             name=f"v2_{i}") for i in range(2)]
            for i in range(2):
                for kv in range(NKV[i]):
                    nc.gpsimd.memset(v2s[i][:, kv, 64:65], 1.0)
                    nc.gpsimd.memset(v2s[i][:, kv, 129:130], 1.0)

            # round-robin psum->sbuf evacuation across engines
            _rr = [0]

            def evac(dst, src):
                e = _rr[0] = (_rr[0] + 1) % 2
                if e == 0:
                    nc.scalar.activation(dst, src, AF.Copy)
                    return nc.scalar
                nc.vector.tensor_copy(dst, src)
                return nc.vector

            def rms_tail(img, co, rstd0, pss, wcol=None,
                         write_back_f32=False):
                csl = slice(co, co + CH)
                act_raw(rstd0[:, csl], pss[:], AF.Rsqrt,
                        bias=epsc[0:1, :], scale=1.0 / HID)
                rstdb = psB.tile([128, 512], f32, tag="psb", name="rstdb")
                nc.tensor.matmul(rstdb[:, 0:CH], lhsT=onesrb[:],
                                 rhs=rstd0[0:1, csl], start=True, stop=True)
                rsb = castp.tile([128, 1, CH], bf16, tag="rsb", bufs=2)
                evac(rsb[:, 0, :], rstdb[:, 0:CH])
                if wcol is None:
                    a1, a2 = bass_mod.broadcast_tensor_aps(
                        resids[img][:, :, csl], rsb[:])
                    nc.vector.tensor_mul(xnorms[img][:, :, csl], a1, a2)
                else:
                    for kt in range(KT):
                        nc.vector.scalar_tensor_tensor(
                            xnorms[img][:, kt, csl],
                            resids[img][:, kt, csl],
                            wcol[:, kt:kt + 1], rsb[:, 0, :],
                            ALU.mult, ALU.mult)
                if write_back_f32:
                    nc.scalar.activation(resids[img][:, :, csl],
                                         xnorms[img][:, :, csl],
                                         AF.Copy)

            def rms_chunk(img, co, rstd0, wcol=None,
                          write_back_f32=False):
                csl = slice(co, co + CH)
                pss = psS.tile([1, CH], f32, tag="pss")
                sq = castp.tile([128, KT, CH], bf16, tag="sq", bufs=1)
                nc.vector.tensor_mul(sq[:, 0:4, :],
                                     resids[img][:, 0:4, csl],
                                     resids[img][:, 0:4, csl])
                nc.vector.tensor_add(sq[:, 0:2, :], sq[:, 0:2, :],
                                     sq[:, 2:4, :])
                nc.vector.tensor_mul(sq[:, 4:8, :],
                                     resids[img][:, 4:8, csl],
                                     resids[img][:, 4:8, csl])
                nc.vector.tensor_add(sq[:, 4:6, :], sq[:, 4:6, :],
                                     sq[:, 6:8, :])
                nc.vector.tensor_add(sq[:, 0:2, :], sq[:, 0:2, :],
                                     sq[:, 4:6, :])
                nc.vector.tensor_add(sq[:, 0, :], sq[:, 0, :], sq[:, 1, :])
                nc.tensor.matmul(pss[:], lhsT=ones1[:], rhs=sq[:, 0, :],
                                 start=True, stop=True)
                rms_tail(img, co, rstd0, pss, wcol, write_back_f32)

            def rms_norm(img, wcol=None, write_back_f32=False):
                W = IMW[img]
                rstd0 = wrk1p.tile([1, W], bf16, tag=f"rstd0{img}")
                for co in range(0, W, CH):
                    rms_chunk(img, co, rstd0, wcol, write_back_f32)

            # ---- conv patch embed (replicated, streamed) + ln_pre ----
            with tc.tile_pool(name="convp", bufs=2) as convp:
                rstd0s = [wrk1p.tile([1, IMW[i]], bf16, tag=f"rstd0{i}",
                                     name=f"rstd0c{i}")
                          for i in range(2)]
                for chi in range(NCH):
                    gco = chi * CH
                    img = 0 if gco < S0 else 1
                    lco = gco - IMO[img]
                    pch = convp.tile([128, PKT, CH], bf16, tag="pch")
                    nc.sync.dma_start(pch[:], patchesT_d[:, :, gco:gco + CH])
                    for kt in range(KT):
                        cwt = convp.tile([128, PKT, 128], bf16, tag="cwt")
                        nc.sync.dma_start(
                            cwt[:], convWT_d[:, :, kt * 128:(kt + 1) * 128])
                        psx = psA.tile([128, CH], f32, tag="psx")
                        for pk in range(PKT):
                            nc.tensor.matmul(
                                psx[:], lhsT=cwt[:, pk, :],
                                rhs=pch[:, pk, :],
                                start=(pk == 0), stop=(pk == PKT - 1))
                        nc.scalar.activation(
                            resids[img][:, kt, lco:lco + CH], psx[:],
                            AF.Copy)
                    # ln_pre for this chunk rides under the next chunk's
                    # conv matmuls instead of running serially after all
                    # of conv
                    rms_chunk(img, lco, rstd0s[img], lnw,
                              write_back_f32=True)

            def qkv_attn(img, wq, wk, wv, wo, pend_in,
                         lnw_post=None):
                """Full attention for one image -> AR output dram tile."""
                W = IMW[img]
                lo = IMO[img]
                nq = W // CH
                nkv = NKV[img]
                v2 = v2s[img]
                qt = attp.tile([128, W], bf16, tag=f"qt{img}")
                kt_t = attp.tile([128, W], bf16, tag=f"kt{img}")
                otcs = [attp.tile([128, CH], bf16, tag=f"otc{img}_{ci}",
                                  name=f"otc{img}_{ci}") for ci in range(nq)]
                arouts = []

                def oproj(ci):
                    # row-parallel o-projection: this core's 2 heads (128
                    # rows) x full o-weight slice -> partial [HID, CH],
                    # AllReduced per chunk
                    arin = dramp.tile([128, KT, CH], bf16,
                                      tag=f"coi{img}{ci}",
                                      name=f"coi{img}{ci}")
                    aro = dramp.tile([128, KT, CH], bf16,
                                     tag=f"coo{img}{ci}",
                                     name=f"coo{img}{ci}",
                                     addr_space="Shared")
                    stage = stgp.tile([128, KT, CH], bf16, tag="stgo",
                                      bufs=1)
                    for kt in range(KT):
                        pso = psA.tile([128, CH], f32, tag="psx")
                        nc.tensor.matmul(pso[:], lhsT=wo[:, kt, :],
                                         rhs=otcs[ci][:],
                                         start=True, stop=True)
                        evac(stage[:, kt, :], pso[:])
                    nc.scalar.dma_start(arin[:], stage[:])
                    nc.gpsimd.collective_compute(
                        "AllReduce", ALU.add,
                        ins=[arin.opt()], outs=[aro.opt()],
                        replica_groups=[list(range(NCORES))])
                    arouts.append((aro, ci * CH))

                def qkrope_chunk(co):
                    # q/k projection + rope + v blocks for ONE token chunk
                    # (lets attention on earlier chunks start while later
                    # chunks still wait on their mlp-AllReduce)
                    csl = slice(co, co + CH)
                    gsl = slice(lo + co, lo + co + CH)
                    for dst, w in ((qt, wq), (kt_t, wk)):
                        psq = psA.tile([128, CH], f32, tag="psx")
                        for kt in range(KT):
                            nc.tensor.matmul(
                                psq[:], lhsT=w[:, kt, :],
                                rhs=xnorms[img][:, kt, csl],
                                start=(kt == 0), stop=(kt == KT - 1))
                        nc.scalar.activation(dst[:, csl], psq[:], AF.Copy)
                    for dst in (qt, kt_t):
                        psr = psB.tile([128, 512], f32, tag="psb")
                        nc.tensor.matmul(psr[:, 0:CH], lhsT=rotP[:],
                                         rhs=dst[:, csl],
                                         start=True, stop=True)
                        t1 = castp.tile([128, CH], bf16, tag="t1", bufs=2)
                        t2 = castp.tile([128, CH], bf16, tag="t2", bufs=2)
                        nc.vector.tensor_mul(t1[:], dst[:, csl],
                                             cos2[:, gsl])
                        nc.vector.tensor_mul(t2[:], psr[:, 0:CH],
                                             sin2[:, gsl])
                        nc.vector.tensor_add(dst[:, csl], t1[:], t2[:])
                    for kv in range(co // 128, (co + CH) // 128):
                        psv = psB.tile([128, 512], f32, tag="psb")
                        for kt in range(KT):
                            nc.tensor.matmul(
                                psv[:, 0:128],
                                lhsT=xnorms[img][:, kt,
                                                 kv * 128:(kv + 1) * 128],
                                rhs=wv[:, kt, :],
                                start=(kt == 0), stop=(kt == KT - 1))
                        nc.vector.tensor_copy(v2[:, kv, 0:64], psv[:, 0:64])
                        nc.scalar.activation(v2[:, kv, 65:129],
                                             psv[:, 64:128], AF.Copy)

                psavs_by = {}
                pts_by = {}

                def issue_scores(ci, i):
                    qsl = slice(ci * CH, (ci + 1) * CH)
                    for h in range(2):
                        hsl = slice(h * 64, (h + 1) * 64)
                        pss = psB.tile([128, 512], f32, tag="psb")
                        nc.tensor.matmul(
                            pss[:, 0:CH],
                            lhsT=kt_t[hsl, i * 128:(i + 1) * 128],
                            rhs=qt[hsl, qsl], start=True, stop=True)
                        pt = castp.tile([128, CH], bf16, tag="pt",
                                        bufs=4, name=f"pt{h}")
                        nc.scalar.activation(pt[:], pss[:, 0:CH],
                                             AF.Exp, scale=SCALE)
                        pts_by[(ci, h, i)] = pt

                def att_seg(ci, a, b):
                    # scores+AV for q-chunk ci against kv blocks [a, b);
                    # the AV psum accumulation stays open across segments
                    if ci not in psavs_by:
                        psavs_by[ci] = [psC.tile([65, CH], f32, tag="psav",
                                                 name=f"psav{h}")
                                        for h in range(2)]
                    psavs = psavs_by[ci]
                    issue_scores(ci, a)
                    for i in range(a, b):
                        if i + 1 < b:
                            issue_scores(ci, i + 1)
                        for h in range(2):
                            nc.tensor.matmul(
                                psavs[h][:],
                                lhsT=v2[:, i, h * 65:h * 65 + 65],
                                rhs=pts_by.pop((ci, h, i))[:],
                                start=(i == 0), stop=(i == nkv - 1))

                def fin(ci):
                    # softmax denominators + output scaling + o-projection
                    psavs = psavs_by.pop(ci)
                    psbc = psB.tile([128, 512], f32, tag="psb")
                    for h in range(2):
                        rec = castp.tile([1, CH], bf16, tag="rec", bufs=2)
                        with nc.allow_low_precision("softmax denom bf16"):
                            nc.vector.reciprocal(rec[:], psavs[h][64:65, :])
                        nc.tensor.matmul(
                            psbc[h * 64:(h + 1) * 64, 0:CH],
                            lhsT=onesr[:], rhs=rec[:],
                            start=True, stop=True, skip_group_check=True)
                    obc = castp.tile([128, CH], bf16, tag="obc", bufs=2)
                    nc.vector.tensor_copy(obc[:], psbc[:, 0:CH])
                    for h in range(2):
                        nc.vector.tensor_mul(
                            otcs[ci][h * 64:(h + 1) * 64, :],
                            psavs[h][0:64, :], obc[h * 64:(h + 1) * 64, :])
                    oproj(ci)

                rstd0 = wrk1p.tile([1, W], bf16, tag=f"rstd0{img}")

                def consume(ci):
                    # consume this chunk's pending mlp-AllReduce (or do the
                    # initial rms) right before the chunk's q/k/v work, so
                    # later chunks' ARs are not in front of earlier chunks'
                    # attention in any queue
                    if pend_in is not None:
                        aro, co = pend_in[ci]
                        consume_chunk(img, aro, co, rstd0)
                    else:
                        rms_chunk(img, ci * CH, rstd0, lnw_post)

                if nq == 1:
                    consume(0)
                    qkrope_chunk(0)
                    att_seg(0, 0, nkv)
                    fin(0)
                else:
                    consume(0)
                    qkrope_chunk(0)
                    att_seg(0, 0, 4)
                    consume(1)
                    qkrope_chunk(CH)
                    att_seg(0, 4, 8)
                    fin(0)
                    att_seg(1, 0, 8)
                    fin(1)
                return arouts

            def consume_chunk(img, aro, co, rstd0, write_out=False):
                lo = IMO[img]
                csl = slice(co, co + CH)
                arr = wrk2p.tile([128, KT, CH], bf16, tag="arrc",
                                 bufs=1)
                H = KT // 2
                nc.sync.dma_start(arr[:, 0:H, :], aro[:, 0:H, :])
                nc.sync.dma_start(arr[:, H:KT, :], aro[:, H:KT, :])
                nc.vector.tensor_add(resids[img][:, 0:H, csl],
                                     resids[img][:, 0:H, csl],
                                     arr[:, 0:H, :])
                nc.vector.tensor_add(resids[img][:, H:KT, csl],
                                     resids[img][:, H:KT, csl],
                                     arr[:, H:KT, :])
                if write_out:
                    nc.sync.dma_start(
                        out_d[:, :, lo + co:lo + co + CH],
                        resids[img][:, :, csl])
                if rstd0 is not None:
                    rms_chunk(img, co, rstd0)

            def add_ar(img, arouts, rms=False, write_out=False):
                W = IMW[img]
                rstd0 = None
                if rms:
                    rstd0 = wrk1p.tile([1, W], bf16, tag=f"rstd0{img}")
                for aro, co in arouts:
                    consume_chunk(img, aro, co, rstd0, write_out)

            def mlp(img, wg, wu, wd, ar_o, final=False):
                W = IMW[img]
                lo = IMO[img]
                arouts = []
                rstd0 = wrk1p.tile([1, W], bf16, tag=f"rstd0{img}")
                for co in range(0, W, CH):
                    ci = co // CH
                    # this chunk's o-AllReduce lands here; later chunks'
                    # o-ARs stay behind this chunk's mlp in every queue
                    aro_o, co_o = ar_o[ci]
                    consume_chunk(img, aro_o, co_o, rstd0)
                    if final:
                        # pre-mlp residual out now (hidden under the mlp
                        # compute); host adds the ReduceScattered delta
                        nc.sync.dma_start(
                            out_d[:, :, lo + co:lo + co + CH],
                            resids[img][:, :, co:co + CH])
                    csl = slice(co, co + CH)
                    for mt in range(MT_I):
                        msl = slice(mt * 128, (mt + 1) * 128)
                        psg = psA.tile([128, CH], f32, tag="psx")
                        for kt in range(KT):
                            nc.tensor.matmul(
                                psg[:], lhsT=wg[:, kt, msl],
                                rhs=xnorms[img][:, kt, csl],
                                start=(kt == 0), stop=(kt == KT - 1))
                        gts = castp.tile([128, CH], bf16, tag="gts")
                        nc.scalar.activation(gts[:], psg[:], AF.Silu)
                        psu = psB.tile([128, 512], f32, tag="psb")
                        for kt in range(KT):
                            nc.tensor.matmul(
                                psu[:, 0:CH], lhsT=wu[:, kt, msl],
                                rhs=xnorms[img][:, kt, csl],
                                start=(kt == 0), stop=(kt == KT - 1))
                        nc.vector.tensor_mul(hmlps[img][mt][:, csl], gts[:],
                                             psu[:, 0:CH])
                    arin = dramp.tile([128, KT, CH], bf16,
                                      tag=f"cmi{img}{ci}",
                                      name=f"cmi{img}{ci}")
                    aro = dramp.tile([128, KT, CH], bf16,
                                     tag=f"cmo{img}{ci}",
                                     name=f"cmo{img}{ci}",
                                     addr_space="Shared")
                    stage = stgp.tile([128, KT, CH], bf16, tag="stg",
                                      bufs=2)
                    for kt in range(KT):
                        psd = psA.tile([128, CH], f32, tag="psx")
                        for mt in range(MT_I):
                            nc.tensor.matmul(
                                psd[:],
                                lhsT=wd[:, mt, kt * 128:(kt + 1) * 128],
                                rhs=hmlps[img][mt][:, co:co + CH],
                                start=(mt == 0), stop=(mt == MT_I - 1))
                        evac(stage[:, kt, :], psd[:])
                    nc.scalar.dma_start(arin[:], stage[:])
                    if final:
                        gi = ci if img == 0 else 2
                        rso = dramp.tile([16, KT, CH], bf16,
                                         tag=f"rso{img}{ci}",
                                         name=f"rso{img}{ci}")
                        nc.gpsimd.collective_compute(
                            "ReduceScatter", ALU.add,
                            ins=[arin.opt()], outs=[rso.opt()],
                            replica_groups=[list(range(NCORES))])
                        nc.sync.dma_start(out2_d[:, gi], rso[:])
                    else:
                        nc.gpsimd.collective_compute(
                            "AllReduce", ALU.add,
                            ins=[arin.opt()], outs=[aro.opt()],
                            replica_groups=[list(range(NCORES))])
                        arouts.append((aro, co))
                return arouts

            # ---- transformer layers, software-pipelined across the MLP
            # AllReduce: layer l's MLP AR for image i is added at the top of
            # layer l+1 right before that image's attention norm. Image 1
            # (the small one) goes first so its AllReduce hides under image
            # 0's larger compute ----
            with tc.tile_pool(name="stg", bufs=2) as stgp:
                wts = {}

                def load_weights(l):
                    # weight DMAs ride the gpsimd queue: it carries only
                    # AR-input writes and collective triggers, so these
                    # never sit behind a blocking AR-output wait
                    wq = watp.tile([128, KT, 128], bf16, tag="wq")
                    wk = watp.tile([128, KT, 128], bf16, tag="wk")
                    wv = watp.tile([128, KT, 128], bf16, tag="wv")
                    wo = watp.tile([128, KT, 128], bf16, tag="wo")
                    wg = wmlpp.tile([128, KT, 512], bf16, tag="wg")
                    wu = wmlpp.tile([128, KT, 512], bf16, tag="wu")
                    wd = wdp.tile([128, MT_I, HID], bf16, tag="wd")
                    for t, d in ((wq, wq_d), (wk, wk_d), (wv, wv_d),
                                 (wo, wo_d), (wg, wg_d), (wu, wu_d),
                                 (wd, wd_d)):
                        nc.sync.dma_start(t[:], d[l])
                    wts[l] = (wq, wk, wv, wo, wg, wu, wd)

                load_weights(0)
                pend = [None, None]
                for l in range(NLAYERS):
                    wq, wk, wv, wo, wg, wu, wd = wts.pop(l)
                    ar_a = [None, None]
                    for img in (1, 0):
                        ar_a[img] = qkv_attn(img, wq, wk, wv, wo,
                                             pend[img])
                        pend[img] = None
                    if l + 1 < NLAYERS:
                        load_weights(l + 1)
                    for img in (1, 0):
                        pend[img] = mlp(img, wg, wu, wd, ar_a[img],
                                        final=(l == NLAYERS - 1))

    nc.compile()
    return nc


# ---------------- host-side prep ----------------

def _im2col(img):
    C, H, W = img.shape
    h, w = H // PATCH, W // PATCH
    p = img.reshape(C, h, PATCH, w, PATCH).transpose(1, 3, 0, 2, 4)
    return p.reshape(h * w, C * PATCH * PATCH)


def _rope_tables():
    freqs = 1.0 / THETA ** (np.arange(0, HD, 2, dtype=np.float64) / HD)
    fh = np.outer(np.arange(MAXSIDE, dtype=np.float64), freqs[::2])
    fw = np.outer(np.arange(MAXSIDE, dtype=np.float64), freqs[1::2])
    pids = np.concatenate([
        (np.arange(h)[:, None] * MAXSIDE + np.arange(w)[None, :]).reshape(-1)
        for h, w in GRIDS])
    inv = np.concatenate([
        np.broadcast_to(fh[:, None, :], (MAXSIDE, MAXSIDE, HD // 4)),
        np.broadcast_to(fw[None, :, :], (MAXSIDE, MAXSIDE, HD // 4))],
        axis=-1).reshape(-1, HD // 2)
    inv = np.concatenate([inv, inv], axis=-1)
    emb = inv[pids]                                   # [S, 64]
    cosT = np.cos(emb).T.astype(np.float32)           # [64, S]
    sinT = np.sin(emb).T.astype(np.float32)
    sinTs = np.concatenate([-sinT[:32], sinT[32:]], axis=0)
    cos2 = np.concatenate([cosT, cosT], axis=0).astype(BF16)
    sin2 = np.concatenate([sinTs, sinTs], axis=0).astype(BF16)
    return np.ascontiguousarray(cos2), np.ascontiguousarray(sin2)


def _rot_perm():
    """rot[m] = q[perm(m)] permutation as a [k, m] matmul constant."""
    P = np.zeros((128, 128), np.float32)
    for b in (0, 64):
        for m in range(32):
            P[b + 32 + m, b + m] = 1.0          # rot[m] = q[m+32]
            P[b + m, b + 32 + m] = 1.0          # rot[m+32] = q[m]
    return P.astype(BF16)


def _ktile(a, last):
    """[L, 1024, last] -> [L, 128, kt, last] (partition-major k-tiles)."""
    L = a.shape[0]
    return np.ascontiguousarray(
        a.reshape(L, -1, 128, last).transpose(0, 2, 1, 3))


def _prep(inputs):
    f32 = np.float32
    patches = np.concatenate([
        _im2col(np.asarray(inputs["img0"], f32)),
        _im2col(np.asarray(inputs["img1"], f32))])          # [S, 768]
    patchesT = np.ascontiguousarray(
        patches.T.reshape(PKT, 128, S).transpose(1, 0, 2)).astype(BF16)
    cw = np.asarray(inputs["conv_w"], f32).reshape(HID, 768)
    convWT = np.ascontiguousarray(
        cw.T.reshape(PKT, 128, HID).transpose(1, 0, 2)).astype(BF16)
    cos2, sin2 = _rope_tables()
    lnw = np.ascontiguousarray(
        np.asarray(inputs["ln_pre_w"], f32).reshape(KT, 128).T)

    anw = np.asarray(inputs["attn_norm_w"], f32)[:, :, None]  # [4, in, 1]
    fnw = np.asarray(inputs["ffn_norm_w"], f32)[:, :, None]
    qwT = np.asarray(inputs["q_w"], f32).transpose(0, 2, 1) * anw
    kwT = np.asarray(inputs["k_w"], f32).transpose(0, 2, 1) * anw
    vwT = np.asarray(inputs["v_w"], f32).transpose(0, 2, 1) * anw
    owT = np.asarray(inputs["o_w"], f32).transpose(0, 2, 1)   # [4, d, e]
    gwT = np.asarray(inputs["gate_w"], f32).transpose(0, 2, 1) * fnw
    uwT = np.asarray(inputs["up_w"], f32).transpose(0, 2, 1) * fnw
    dwT = np.asarray(inputs["down_w"], f32).transpose(0, 2, 1)  # [4, I, out]

    common = dict(patchesT=patchesT, convWT=convWT, cos2=cos2, sin2=sin2,
                  rotP=_rot_perm(), lnw=lnw)
    in_maps = []
    for c in range(NCORES):
        esl = slice(c * 128, (c + 1) * 128)
        isl = slice(c * 512, (c + 1) * 512)
        m = dict(
            wq=_ktile(qwT[:, :, esl].astype(BF16), 128),
            wk=_ktile(kwT[:, :, esl].astype(BF16), 128),
            wv=_ktile(vwT[:, :, esl].astype(BF16), 128),
            wo=np.ascontiguousarray(
                owT[:, esl, :].reshape(NLAYERS, 128, KT, 128)).astype(BF16),
            wg=_ktile(gwT[:, :, isl].astype(BF16), 512),
            wu=_ktile(uwT[:, :, isl].astype(BF16), 512),
            wd=np.ascontiguousarray(
                dwT[:, isl, :].reshape(NLAYERS, MT_I, 128, HID)
                .transpose(0, 2, 1, 3)).astype(BF16),
            **common)
        in_maps.append(m)
    return in_maps


LAST_RESULTS = None
TRACE = False


def _install_ntff_hook():
    """The RL container's antenv lacks axon_hooks; recreate it so
    trace=True can capture NTFF profiles through the axon terminal."""
    import types
    import antenv

    if hasattr(antenv, "axon_hooks"):
        return
    mod = types.ModuleType("antenv.axon_hooks")
    holder = [None]
    mod.set_axon_ntff_profile_hook = lambda h: holder.__setitem__(0, h)
    mod.get_axon_ntff_profile_hook = lambda: holder[0]
    sys.modules["antenv.axon_hooks"] = mod
    antenv.axon_hooks = mod
    if "/root/.axon_site" not in sys.path:
        sys.path.insert(0, "/root/.axon_site")
    try:
        from trn_agent_boot.trn_boot import _ntff_profile_via_ctypes
        mod.set_axon_ntff_profile_hook(
            _ntff_profile_via_ctypes("/opt/axon/libaxon_pjrt.so"))
    except Exception as e:  # pragma: no cover
        print("ntff hook install failed:", e)


def kernel(**inputs):
    global LAST_RESULTS
    from concourse import bass_utils

    if TRACE:
        _install_ntff_hook()

    if "nc" not in _CACHE:
        _CACHE["nc"] = _build_nc()
    nc = _CACHE["nc"]
    in_maps = _prep(inputs)
    res = bass_utils.run_bass_kernel_spmd(
        nc, in_maps, core_ids=list(range(NCORES)), trace=TRACE)
    LAST_RESULTS = res
    out = res.results[0]["out"]          # [128, KT, S] pre-mlp residual
    full = out.transpose(1, 0, 2).reshape(HID, S).astype(np.float32)
    # final-layer mlp delta arrives ReduceScattered: core c holds hidden
    # rows [c*16, (c+1)*16) of every k-tile
    d = np.zeros((KT, 128, NCH * CH), np.float32)
    for c in range(NCORES):
        o2 = np.asarray(res.results[c]["out2"], np.float32)
        d[:, c * 16:(c + 1) * 16] = (
            o2.transpose(2, 0, 1, 3).reshape(KT, 16, NCH * CH))
    full += d.reshape(HID, NCH * CH)
    return np.ascontiguousarray(full.T[None]).astype(np.float32)

